# revision 24
# baseline (speedup 1.0000x reference)
"""Banded DTW loss kernel for Trainium2 (Bass/Tile), 8-core data-parallel.

Meet-in-the-middle restructure of the row-serial DP so the DVE engine
pipelines two independent dependency chains (the Tile sem chain then only
forces each instruction to wait two-back, hiding the ~95ns completion
latency of its predecessor):

  Phase A: forward DP over rows 0..511 (chain F) runs interleaved with a
           backward DP over rows 1023..512 (chain B).  The backward DP in
           flipped band coordinates (o -> 40-o, rows renumbered from the
           end) has the exact forward recurrence, so both chains are the
           same (min, tensor_tensor_scan) pair per row.
  Crossing: the optimal path crosses rows 511->512 once; argmin over the
           41 candidate transitions of Df[511] + min(Db[512] up, diag)
           seeds both walks (ties are measure-zero with random inputs).
  Phase B: per-cell backtrack choice bits + per-row g/L scans, uniform
           over the combined slot layout (forward rows, a virtual seam
           row, backward rows).
  Walks:   two independent 511-step walks (lower backtrack from the
           crossing, upper forward-track in flipped space), interleaved.
  Metrics: interval masks + the four path aggregates, one bulk pass.

Slot layout r' in [0,1087] (34 blocks x 32, p=r'%32, b=r'//32):
  r' in [1,512]    forward row i = r'-1
  r' = 0, 513      virtual DP origin rows
  r' in [514,1025] backward row rt = r'-513, original row i = 1537-r',
                   band flipped (cell col c corresponds to o = 41-c)
  r' >= 1026       junk (masked)

Band-validity BIG costs are folded into the host-prepped skewed targets
(invalid cells get tx=ty=1e29 so d ~ 2e29 there), removing the on-device
validity mask ops entirely.

Sharding: batch 32 -> 4 samples per core on 8 cores; host sums partials.
"""

import numpy as np

import concourse.bacc as bacc
import concourse.bass as bass
import concourse.mybir as mybir
import concourse.tile as tile
from concourse.bass_utils import run_bass_kernel_spmd

B, N, NF = 32, 1024, 4
W = 20
NCORES = 8
BC = B // NCORES          # samples per core
BIG = 1e30
BIGD = 1e29               # folded invalid-cell target value
NB = 41                   # band width
CW = 43                   # RE row width (col 0 pad, col c=o+1, col 42 pad)
NBLK = 34                 # RE blocks
NSLOT = NBLK * 32         # 1088
RE = NBLK * CW            # 1462
NRING = 64                # ring/window depth per chain
HROWS = 512               # rows per chain

AL = mybir.AluOpType
DT = mybir.dt.float32

# ---- megaQ ([128, QW]) column offsets ----
WINF_O = 0                          # chain F rolling window, 64 x 42
VRF_O = WINF_O + NRING * 42         # virtual row (42)
RINGF_O = VRF_O + 42                # chain F d ring, 64 x 41
TMPF_O = RINGF_O + NRING * NB       # chain F scratch (48)
WINB_O = TMPF_O + 48
VRB_O = WINB_O + NRING * 42
RINGB_O = VRB_O + 42
TMPB_O = RINGB_O + NRING * NB
CRS_O = TMPB_O + 48                 # crossing scratch (64)
WSCL_O = CRS_O + 64                 # lower walk scratch (48)
XHL_O = WSCL_O + 48                 # @0: x cols for slots 0..543
GWLO_O = XHL_O + 544                # @0: g maps, raw 43-wide, (p,b)-major
QW_LO = GWLO_O + 32 * 17 * CW
QW = QW_LO
# The upper walk lives on partitions 4:8 at the SAME columns as the lower
# walk (GWLO/XHL/WSCL): pairing lower slot s with upper slot s+513 makes
# every per-step column identical, so one 8-partition op steps both walks.

# ---- megaRE ([128, 8*RE + small]) regions ----
R1_O, R2_O, R3_O, R4_O, R5_O, R6_O, R7_O, R8_O = (i * RE for i in range(8))
SM_O = 8 * RE
PX_O, PY_O, PZ_O = SM_O, SM_O + NBLK, SM_O + 2 * NBLK
XC_O, OLO_O = SM_O + 3 * NBLK, SM_O + 4 * NBLK
CST0_O = SM_O + 5 * NBLK            # cst cols; COLIO = CST0+1 holds value 0
COLIO_O = CST0_O + 1
CLZ_O = CST0_O + 48
SPZ_O, SPN_O, QZ_O, NGZ_O = (CLZ_O + k * NBLK for k in (1, 2, 3, 4))
RED_O = CLZ_O + 5 * NBLK            # Sx, Sy, Sbce, cnt
REW = RED_O + 8

_CACHE = {}


def _manual_ap(base, extra_off, dims):
    """AP with base's partition dim and explicit free [stride, count] dims."""
    ap0 = [list(base.ap[0])]
    return bass.AP(base.tensor, base.offset + extra_off,
                   ap0 + [list(d) for d in dims])


def _pstride_ap(base, pstep, pcount, dims):
    """AP stepping pstep partitions at a time (base at start partition)."""
    return bass.AP(base.tensor, base.offset,
                   [[pstep * base.ap[0][0], pcount]] + [list(d) for d in dims])


def _build_module():
    nc = bacc.Bacc("TRN2", target_bir_lowering=False, debug=False,
                   num_devices=NCORES)
    pre = nc.dram_tensor("pre", [128, 3 * NBLK], DT, kind="ExternalInput")
    skw = nc.dram_tensor("skw", [128, 3 * RE], DT, kind="ExternalInput")
    cst = nc.dram_tensor("cst", [128, 48], DT, kind="ExternalInput")
    partials = nc.dram_tensor("partials", [128, 4], DT, kind="ExternalOutput")
    dram_d = nc.dram_tensor("dscr_d", [BC, NSLOT, NB], DT, kind="Internal")
    dram_D = nc.dram_tensor("dscr_D", [BC, NSLOT + 1, NB], DT, kind="Internal")

    with tile.TileContext(nc) as tc:
        with tc.tile_pool(name="main", bufs=1) as pool:
            megaQ = pool.tile([128, QW], DT)
            megaRE = pool.tile([128, REW], DT)
            _emit(nc, megaQ, megaRE, pre, skw, cst, partials, dram_d, dram_D)
    nc.compile()
    return nc


def _emit(nc, megaQ, megaRE, pre, skw, cst, partials, dram_d, dram_D):
    v = nc.vector

    def cells(off, dc=0):
        """[128, 34, 41] view of RE region cols (b*43 + 1 + dc)."""
        return megaRE[:, off:off + RE].rearrange(
            "p (b c) -> p b c", c=CW)[:, :, 1 + dc:NB + 1 + dc]

    def reblk(off, b, dc=0, w=NB):
        s = off + b * CW + 1 + dc
        return megaRE[:, s:s + w]

    def smb(off):
        """[128, 34] small block broadcast over the 41 band cols."""
        return megaRE[:, off:off + NBLK].unsqueeze(2).broadcast_to([128, NBLK, NB])

    # ---------------- input DMAs ----------------
    nc.sync.dma_start(out=megaRE[:, CST0_O:CST0_O + 48], in_=cst[:])
    nc.sync.dma_start(out=megaRE[:, PX_O:PX_O + 3 * NBLK], in_=pre[:])
    nc.sync.dma_start(out=megaRE[:, R7_O:R7_O + RE], in_=skw[:, 0:RE])
    nc.sync.dma_start(out=megaRE[:, R8_O:R8_O + RE], in_=skw[:, RE:2 * RE])
    nc.scalar.dma_start(out=megaRE[:, R3_O:R3_O + RE], in_=skw[:, 2 * RE:3 * RE])

    # ---------------- init memsets ----------------
    for win_o, vr_o in ((WINF_O, VRF_O), (WINB_O, VRB_O)):
        nc.gpsimd.memset(megaQ[0:4, win_o:win_o + NRING * 42], BIG)
        nc.gpsimd.memset(megaQ[0:4, vr_o:vr_o + 42], BIG)
        nc.gpsimd.memset(megaQ[0:4, vr_o + W:vr_o + W + 1], 0.0)
    nc.gpsimd.memset(megaQ[0:8, XHL_O:XHL_O + 544], 0.0)   # xh arrays (junk=0)
    v.memset(megaRE[:, XC_O:XC_O + NBLK], 0.0)             # junk XC = 0
    nc.gpsimd.memset(megaRE[:, R4_O:R4_O + RE], BIG)
    nc.gpsimd.memset(megaRE[0:4, R4_O + 21:R4_O + 22], 0.0)    # slot 0 origin
    # wait: slot-0 origin lives at partitions {32s}, not 0:4 -- write all 4
    for s in range(1, BC):
        nc.sync.dma_start(
            out=megaRE[32 * s:32 * s + 1, R4_O + 21:R4_O + 22],
            in_=megaRE[0:1, R4_O + 21:R4_O + 22])
    # slot 513 origin at partitions {32s+1} (DMA: non-quadrant start)
    for s in range(BC):
        nc.sync.dma_start(
            out=megaRE[32 * s + 1:32 * s + 2,
                       R4_O + 16 * CW + 21:R4_O + 16 * CW + 22],
            in_=megaRE[0:1, R4_O + 21:R4_O + 22])

    # ---------------- d build ----------------
    # d = |px - tx| + |py - ty|; invalid cells come out ~2e29 automatically.
    v.tensor_tensor(out=cells(R5_O), in0=smb(PX_O), in1=cells(R7_O),
                    op=AL.subtract)
    nc.scalar.activation(cells(R1_O), cells(R5_O),
                         mybir.ActivationFunctionType.Abs)
    v.tensor_tensor(out=cells(R6_O), in0=smb(PY_O), in1=cells(R8_O),
                    op=AL.subtract)
    nc.scalar.activation(cells(R2_O), cells(R6_O),
                         mybir.ActivationFunctionType.Abs)
    v.tensor_tensor(out=cells(R7_O), in0=cells(R1_O), in1=cells(R2_O),
                    op=AL.add)

    nc.gpsimd.memset(megaRE[:, R5_O:R5_O + RE], BIG)   # DrePrev pads

    # ---------------- stage d to DRAM (slot layout) ----------------
    df = dram_d[:]
    Df = dram_D[:]
    for b0, b1 in ((0, 18), (18, NBLK)):
        for s in range(BC):
            src = _manual_ap(megaRE[32 * s:32 * s + 32,
                                    R7_O + b0 * CW + 1:R7_O + b0 * CW + 2],
                             0, [[CW, b1 - b0], [1, NB]])
            dst = bass.AP(df.tensor, s * NSLOT * NB + b0 * 32 * NB,
                          [[NB, 32], [32 * NB, b1 - b0], [1, NB]])
            nc.sync.dma_start(out=dst, in_=src)

    # ---------------- seed dram_D with the initialized R4 ----------------
    # (virtual origin rows + BIG everywhere else; chunk stores overwrite the
    # real slots, so the bulk reload after the chains restores a consistent
    # picture including the origins)
    for s in range(BC):
        dsrc = _manual_ap(megaRE[32 * s:32 * s + 32, R4_O + 1:R4_O + 2],
                          0, [[CW, NBLK], [1, NB]])
        ddst = bass.AP(Df.tensor, (s * (NSLOT + 1) + 1) * NB,
                       [[NB, 32], [32 * NB, NBLK], [1, NB]])
        nc.scalar.dma_start(out=ddst, in_=dsrc)
        nc.scalar.dma_start(
            out=bass.AP(Df.tensor, s * (NSLOT + 1) * NB, [[1, NB]]),
            in_=megaRE[32 * s:32 * s + 1, R4_O + 1:R4_O + 1 + NB])

    # ---------------- phase A: two interleaved DP chains ----------------
    qbase = megaQ[0:4, 0:1]
    chains = (
        dict(win=WINF_O, vr=VRF_O, ring=RINGF_O, tmp=TMPF_O, slot0=1),
        dict(win=WINB_O, vr=VRB_O, ring=RINGB_O, tmp=TMPB_O, slot0=514),
    )

    def ring_load(ch, k):
        c = (k - 1) // 32
        slot0 = ch["slot0"] + 32 * c
        rdst = _manual_ap(qbase, ch["ring"] + (((k - 1) % NRING)) * NB,
                          [[NB, 32], [1, NB]])
        rsrc = bass.AP(df.tensor, slot0 * NB,
                       [[NSLOT * NB, BC], [NB, 32], [1, NB]])
        nc.gpsimd.dma_start(out=rdst, in_=rsrc)

    def chunk_out(ch, k):
        c = (k - 1) // 32
        slot0 = ch["slot0"] + 32 * c
        wsrc = _manual_ap(qbase, ch["win"] + ((32 * c) % NRING) * 42,
                          [[42, 32], [1, NB]])
        wdst = bass.AP(Df.tensor, (slot0 + 1) * NB,
                       [[(NSLOT + 1) * NB, BC], [NB, 32], [1, NB]])
        nc.sync.dma_start(out=wdst, in_=wsrc)

    def reload_piece(b0, b1, which):
        # R4 rows come from dram rows 32b+p+1 (slot data), R5 from 32b+p
        # (previous slot); piece [b0,b1) is complete once the chunk stores
        # covering its slots have run (Tile orders via the dram_D ranges).
        for s in range(BC):
            off = 1 if which == "r4" else 0
            reg = R4_O if which == "r4" else R5_O
            srcp = bass.AP(Df.tensor,
                           (s * (NSLOT + 1) + 32 * b0 + off) * NB,
                           [[NB, 32], [32 * NB, b1 - b0], [1, NB]])
            dstp = _manual_ap(
                megaRE[32 * s:32 * s + 32, reg + b0 * CW + 1:reg + b0 * CW + 2],
                0, [[CW, b1 - b0], [1, NB]])
            qq = (nc.sync, nc.gpsimd, nc.scalar)[s % 3]
            qq.dma_start(out=dstp, in_=srcp)

    for k in range(1, HROWS + 1):
        if k % 32 == 1:
            for ch in chains:
                ring_load(ch, k)
        if k == 384:
            # chunks 0..11 stored: blocks 0..9 and 17..25 are complete in
            # dram_D (block 16 contains slot 512 = the last forward store,
            # so it stays in the tail); reload the early pieces now
            reload_piece(0, 10, "r4")
            reload_piece(17, 26, "r4")
            reload_piece(0, 10, "r5")
            reload_piece(17, 26, "r5")
        for ch in chains:
            wp = ch["vr"] if k == 1 else ch["win"] + ((k - 2) % NRING) * 42
            v.tensor_tensor(out=megaQ[0:4, ch["tmp"]:ch["tmp"] + NB],
                            in0=megaQ[0:4, wp:wp + NB],
                            in1=megaQ[0:4, wp + 1:wp + NB + 1], op=AL.min)
        for ch in chains:
            ws = ch["win"] + ((k - 1) % NRING) * 42
            rg = ch["ring"] + ((k - 1) % NRING) * NB
            v.tensor_tensor_scan(out=megaQ[0:4, ws:ws + NB],
                                 data0=megaQ[0:4, ch["tmp"]:ch["tmp"] + NB],
                                 data1=megaQ[0:4, rg:rg + NB],
                                 initial=BIG, op0=AL.min, op1=AL.add)
        if k % 32 == 0:
            for ch in chains:
                chunk_out(ch, k)

    # -------- reload tails (blocks not covered by the k==384 pieces) --------
    reload_piece(10, 17, "r4")
    reload_piece(26, NBLK, "r4")
    reload_piece(10, 17, "r5")
    reload_piece(26, NBLK, "r5")

    # ---------------- crossing ----------------
    wfr = WINF_O + ((HROWS - 1) % NRING) * 42
    wbr = WINB_O + ((HROWS - 1) % NRING) * 42
    rev41 = _manual_ap(qbase, wbr + 41, [[-1, NB]])
    rev40 = _manual_ap(qbase, wbr + 40, [[-1, NB]])
    t1 = megaQ[0:4, CRS_O:CRS_O + NB]
    mv = megaQ[0:4, CRS_O + 44:CRS_O + 45]
    ups = megaQ[0:4, CRS_O + 45:CRS_O + 46]
    seedU = megaQ[0:4, CRS_O + 46:CRS_O + 47]
    sl2 = megaQ[0:4, CRS_O + 47:CRS_O + 48]
    iot0 = megaRE[0:4, COLIO_O + 1:COLIO_O + 1 + NB]
    v.tensor_tensor(out=t1, in0=rev41, in1=rev40, op=AL.min)
    v.tensor_tensor(out=t1, in0=megaQ[0:4, wfr:wfr + NB], in1=t1, op=AL.add)
    v.tensor_reduce(out=mv, in_=t1, axis=mybir.AxisListType.X, op=AL.min)
    v.scalar_tensor_tensor(out=megaQ[0:4, TMPF_O:TMPF_O + NB], in0=t1,
                           scalar=mv, in1=iot0, op0=AL.is_equal, op1=AL.mult)
    v.tensor_reduce(out=sl2, in_=megaQ[0:4, TMPF_O:TMPF_O + NB],
                    axis=mybir.AxisListType.X, op=AL.max)
    v.tensor_single_scalar(out=megaQ[0:4, XHL_O + 16:XHL_O + 17], in_=sl2,
                           scalar=0.0, op=AL.add)              # lower seed
    v.tensor_tensor(out=t1, in0=rev41, in1=rev40, op=AL.is_le)
    v.scalar_tensor_tensor(out=megaQ[0:4, TMPF_O:TMPF_O + NB], in0=iot0,
                           scalar=sl2, in1=t1, op0=AL.is_equal, op1=AL.mult,
                           accum_out=ups)
    # seedU = 42 - seedL + upsel
    v.tensor_scalar(out=seedU, in0=sl2, scalar1=-1.0, scalar2=42.0,
                    op0=AL.mult, op1=AL.add)
    v.tensor_tensor(out=seedU, in0=seedU, in1=ups, op=AL.add)
    # upper-walk seed (slot 1025 -> idx 16) on partitions 4:8
    nc.sync.dma_start(out=megaQ[4:8, XHL_O + 16:XHL_O + 17],
                      in_=_manual_ap(megaQ[0:4, CRS_O + 46:CRS_O + 47],
                                     0, [[1, 1]]))

    # ---------------- phase B: choice bits + g/L scans ----------------
    diag, up = cells(R5_O, 0), cells(R5_O, 1)
    left = cells(R4_O, -1)
    v.tensor_tensor(out=cells(R6_O), in0=diag, in1=up, op=AL.min)
    v.tensor_tensor(out=cells(R7_O), in0=left, in1=cells(R6_O),
                    op=AL.is_lt)                       # isleft
    v.tensor_tensor(out=cells(R6_O), in0=up, in1=left, op=AL.min)
    v.tensor_tensor(out=cells(R8_O), in0=diag, in1=cells(R6_O),
                    op=AL.is_le)                       # isdiag
    v.tensor_single_scalar(out=cells(R6_O), in_=cells(R7_O),
                           scalar=0.0, op=AL.is_equal)  # notleft
    ocp1 = megaRE[:, COLIO_O + 2:COLIO_O + 2 + NB].unsqueeze(1) \
        .broadcast_to([128, NBLK, NB])
    ocol = megaRE[:, COLIO_O + 1:COLIO_O + 1 + NB].unsqueeze(1) \
        .broadcast_to([128, NBLK, NB])
    v.tensor_tensor(out=cells(R8_O), in0=ocp1, in1=cells(R8_O), op=AL.subtract)
    v.tensor_tensor(out=cells(R8_O), in0=cells(R8_O), in1=cells(R6_O),
                    op=AL.mult)                        # gval
    v.tensor_tensor(out=cells(R6_O), in0=ocol, in1=cells(R6_O), op=AL.mult)  # Lval
    for b in range(NBLK):
        v.tensor_tensor_scan(out=reblk(R5_O, b), data0=reblk(R7_O, b),
                             data1=reblk(R8_O, b), initial=0.0,
                             op0=AL.mult, op1=AL.add)  # g -> R5

    # ---------------- gwalk copies (batched, 8 DMAs) ----------------
    for s in range(BC):
        src = megaRE[32 * s:32 * s + 32, R5_O:R5_O + 17 * CW]
        dst = _manual_ap(megaQ[s:s + 1, GWLO_O:GWLO_O + 1], 0,
                         [[17 * CW, 32], [1, 17 * CW]])
        (nc.sync if s % 2 == 0 else nc.gpsimd).dma_start(out=dst, in_=src)
    for s in range(BC):
        # upper g map for slot su lands at the lower-walk column of su-513:
        # rows p>=1 -> consecutive idx blocks; row p=0 wraps to idx 526
        src = megaRE[32 * s + 1:32 * s + 32, R5_O + 16 * CW:R5_O + 33 * CW]
        dst = _manual_ap(megaQ[4 + s:5 + s, GWLO_O:GWLO_O + 1], 0,
                         [[17 * CW, 31], [1, 17 * CW]])
        (nc.sync if s % 2 == 0 else nc.gpsimd).dma_start(out=dst, in_=src)
        src0 = megaRE[32 * s:32 * s + 1, R5_O + 16 * CW:R5_O + 33 * CW]
        dst0 = megaQ[4 + s:5 + s,
                     GWLO_O + 526 * CW:GWLO_O + 526 * CW + 17 * CW]
        (nc.sync if s % 2 == 0 else nc.gpsimd).dma_start(out=dst0, in_=src0)

    for b in range(NBLK):
        v.tensor_tensor_scan(out=reblk(R8_O, b), data0=reblk(R7_O, b),
                             data1=reblk(R6_O, b), initial=0.0,
                             op0=AL.mult, op1=AL.add)  # L -> R8

    # bce cell values (R6 is dead here; overlaps the walk's latency chain)
    v.tensor_scalar(out=megaRE[:, CLZ_O:CLZ_O + NBLK],
                    in0=megaRE[:, PZ_O:PZ_O + NBLK],
                    scalar1=-4.0, scalar2=4.0, op0=AL.max, op1=AL.min)
    nc.scalar.activation(megaRE[:, NGZ_O:NGZ_O + NBLK],
                         megaRE[:, CLZ_O:CLZ_O + NBLK],
                         mybir.ActivationFunctionType.Exp)
    nc.scalar.activation(megaRE[:, SPZ_O:SPZ_O + NBLK],
                         megaRE[:, NGZ_O:NGZ_O + NBLK],
                         mybir.ActivationFunctionType.Ln, bias=1.0)
    nc.scalar.activation(megaRE[:, NGZ_O:NGZ_O + NBLK],
                         megaRE[:, CLZ_O:CLZ_O + NBLK],
                         mybir.ActivationFunctionType.Exp, scale=-1.0)
    nc.scalar.activation(megaRE[:, SPN_O:SPN_O + NBLK],
                         megaRE[:, NGZ_O:NGZ_O + NBLK],
                         mybir.ActivationFunctionType.Ln, bias=1.0)
    v.scalar_tensor_tensor(out=megaRE[:, QZ_O:QZ_O + NBLK],
                           in0=megaRE[:, SPN_O:SPN_O + NBLK], scalar=5.0,
                           in1=megaRE[:, SPZ_O:SPZ_O + NBLK],
                           op0=AL.mult, op1=AL.subtract)
    v.tensor_tensor(out=cells(R6_O), in0=cells(R3_O), in1=smb(QZ_O), op=AL.mult)
    v.tensor_tensor(out=cells(R6_O), in0=cells(R6_O), in1=smb(SPZ_O), op=AL.add)

    # -------- combined walk: one 8-partition op steps both chains --------
    wsc8 = megaQ[0:8, WSCL_O:WSCL_O + NB]
    iot8 = megaRE[0:8, COLIO_O + 1:COLIO_O + 1 + NB]

    def xlo(slot):
        return XHL_O + (slot % 32) * 17 + slot // 32

    for j in range(HROWS - 1):
        s = HROWS - j                  # lower slots 512..2 (upper: s+513)
        glo = GWLO_O + ((s % 32) * 17 + s // 32) * CW + 1
        g8 = megaQ[0:8, glo:glo + NB]
        v.scalar_tensor_tensor(out=wsc8, in0=iot8,
                               scalar=megaQ[0:8, xlo(s):xlo(s) + 1],
                               in1=g8, op0=AL.is_equal, op1=AL.mult,
                               accum_out=megaQ[0:8, xlo(s - 1):xlo(s - 1) + 1])

    # ---------------- xcol copies (batched, 8 DMAs; LO then HI) ----------------
    for s in range(BC):
        src = _manual_ap(megaQ[s:s + 1, XHL_O:XHL_O + 1], 0, [[17, 32], [1, 16]])
        dst = megaRE[32 * s:32 * s + 32, XC_O:XC_O + 16]
        (nc.sync if s % 2 == 0 else nc.scalar).dma_start(out=dst, in_=src)
    for s in range(BC):
        src = _manual_ap(megaQ[4 + s:5 + s, XHL_O:XHL_O + 1], 0,
                         [[17, 31], [1, 17]])
        dst = megaRE[32 * s + 1:32 * s + 32, XC_O + 16:XC_O + 17 + 16]
        nc.sync.dma_start(out=dst, in_=src)
        src0 = megaQ[4 + s:5 + s, XHL_O + 527:XHL_O + 527 + 16]
        dst0 = megaRE[32 * s:32 * s + 1, XC_O + 17:XC_O + 17 + 16]
        nc.sync.dma_start(out=dst0, in_=src0)
        # slot 512 (p=0, b=16): its hi col is the lower-walk seed
        nc.sync.dma_start(out=megaRE[32 * s:32 * s + 1, XC_O + 16:XC_O + 17],
                          in_=megaQ[s:s + 1, XHL_O + 16:XHL_O + 17])

    # ---------------- lo extraction + masks ----------------
    xcolb = smb(XC_O)
    v.tensor_tensor(out=cells(R7_O), in0=ocol, in1=xcolb, op=AL.is_equal)
    v.tensor_tensor(out=cells(R7_O), in0=cells(R7_O), in1=cells(R8_O), op=AL.mult)
    v.tensor_reduce(out=megaRE[:, OLO_O:OLO_O + NBLK], in_=cells(R7_O),
                    axis=mybir.AxisListType.X, op=AL.add)
    v.tensor_tensor(out=cells(R6_O), in0=ocol, in1=smb(OLO_O), op=AL.is_ge)
    v.tensor_tensor(out=cells(R7_O), in0=ocol, in1=xcolb, op=AL.is_le)
    v.tensor_tensor(out=cells(R5_O), in0=cells(R6_O), in1=cells(R7_O),
                    op=AL.mult)                        # mask
    # virtual/junk slots have XC = 0, so (ocol <= 0) already zeroes them

    # ---------------- metrics (fused multiply+accumulate) ----------------
    for src_o, red in ((R1_O, 0), (R2_O, 1), (R6_O, 2)):
        v.scalar_tensor_tensor(out=cells(R7_O), in0=cells(src_o), scalar=1.0,
                               in1=cells(R5_O), op0=AL.mult, op1=AL.mult,
                               accum_out=megaRE[:, RED_O + red:RED_O + red + 1])
    v.tensor_reduce(out=megaRE[:, RED_O + 3:RED_O + 4], in_=cells(R5_O),
                    axis=mybir.AxisListType.XY, op=AL.add)

    nc.sync.dma_start(out=partials[:], in_=megaRE[:, RED_O:RED_O + 4])


def _get_module():
    if "nc" not in _CACHE:
        _CACHE["nc"] = _build_module()
    return _CACHE["nc"]


def _slot_maps():
    """Per-slot (i, flip) maps for the combined layout."""
    slots = np.arange(NSLOT)
    i_of = np.full(NSLOT, -1, np.int64)
    flip = np.zeros(NSLOT, bool)
    lo = (slots >= 1) & (slots <= 512)
    i_of[lo] = slots[lo] - 1
    hi = (slots >= 514) & (slots <= 1025)
    i_of[hi] = 1537 - slots[hi]
    flip[hi] = True
    return i_of, flip


def _make_inmaps(preds, targs):
    preds = np.ascontiguousarray(preds, dtype=np.float32)
    targs = np.ascontiguousarray(targs, dtype=np.float32)
    i_of, flip = _slot_maps()
    real = i_of >= 0
    oo = np.arange(NB)
    # j index per (slot, o-col): j = i + o - W (fwd), i + (40-o) - W (flipped)
    ob = np.where(flip[:, None], 40 - oo[None, :], oo[None, :])
    jj = i_of[:, None] + ob - W                              # [NSLOT, NB]
    valid = real[:, None] & (jj >= 0) & (jj < N) & (np.abs(jj - i_of[:, None]) <= W)
    jc = np.clip(jj, 0, N - 1)

    cstv = np.zeros((128, 48), dtype=np.float32)
    cstv[:, 1:45] = np.arange(44)[None, :]

    pp = np.arange(32)
    bb = np.arange(NBLK)
    slot_pb = pp[:, None] + 32 * bb[None, :]                 # [32, 34]

    in_maps = []
    for c in range(NCORES):
        ps = preds[c * BC:(c + 1) * BC]
        ts = targs[c * BC:(c + 1) * BC]
        # sample-major partitions: partition index = 32*s + p
        prev = np.zeros((BC, 32, 3 * NBLK), dtype=np.float32)
        skwv = np.zeros((BC, 32, 3, NBLK, CW), dtype=np.float32)
        for k in range(3):
            vv = np.where(real[slot_pb], ps[:, :, k][:, np.clip(i_of[slot_pb], 0, N - 1)], 0.0)
            prev[:, :, k * NBLK:(k + 1) * NBLK] = vv
            tv = ts[:, :, k][:, jc[slot_pb]]                 # [BC, 32, 34, 41]
            fill = BIGD if k < 2 else 0.0
            tv = np.where(valid[slot_pb][None], tv, fill)
            skwv[:, :, k, :, 1:1 + NB] = tv
        in_maps.append({"pre": prev.reshape(128, 3 * NBLK),
                        "skw": skwv.reshape(128, 3 * RE),
                        "cst": cstv})
    return in_maps


def _reduce_host(parts_list, subcoef):
    c0, c1 = float(subcoef[0]), float(subcoef[1])
    loss = 0.0
    for parts in parts_list:
        m = parts.reshape(BC, 32, 4).sum(axis=1)        # [BC, (Sx,Sy,Sb,cnt)]
        for s in range(BC):
            sx, sy, sb, cnt = (float(m[s, k]) for k in range(4))
            loss += c0 * sx + c1 * sy + 0.1 * sb / cnt
    return np.float32(loss)


def run(preds, targs, subcoef, trace=False):
    nc = _get_module()
    in_maps = _make_inmaps(preds, targs)
    res = run_bass_kernel_spmd(nc, in_maps, core_ids=list(range(NCORES)),
                               trace=trace)
    parts = [r["partials"] for r in res.results]
    return _reduce_host(parts, np.asarray(subcoef)), res


def kernel(preds, targs, subcoef):
    out, _ = run(preds, targs, subcoef)
    return out
